# revision 1
# baseline (speedup 1.0000x reference)
"""GPT-Neo (6-layer, hidden 1024, seq 2048) forward pass on 8 TRN2 NeuronCores.

Sharding: sequence-parallel transformer (256 tokens/core) with per-layer
AllGather of K/V; attention in transposed-score orientation with max-free
softmax (scores are small at init scale) and additive causal/window masks fed
as per-core data; vocab-sharded tied-lm-head GEMM at the end (logits computed
transposed, [vocab_shard, 2048] per core, unsharded on host).

Numerics: fp16 operands for all GEMMs (fp32 PSUM), f32 residual stream and
attention weights (exp kept in f32; ctx/row-sum matmuls in fp32), layernorm
affine params and all biases folded into weights/bias-vectors host-side.
"""
import sys
import numpy as np

sys.path.insert(0, "/opt/trn_rl_repo")

import concourse.bass as bass
import concourse.tile as tile
from concourse import mybir, bacc
from concourse.bass_utils import run_bass_kernel_spmd
from concourse.masks import make_identity

NCORES = 8
T = 2048
TL = T // NCORES   # 256 tokens per core
H = 1024
HEADS = 16
HD = 64
MLP = 4096
NL = 6
WINDOW = 256
VOCAB = 50257
VSH = 6400         # padded per-core vocab shard (8*6400 = 51200)
EPS = 1e-5
ATTN_LOCAL = [False, True, False, True, False, True]

F16 = mybir.dt.float16
F32 = mybir.dt.float32
BF16 = mybir.dt.bfloat16

KB = T // 128      # 16 key blocks
HP = HEADS // 2    # 8 head pairs
RG = [list(range(NCORES))]


def build(n_layers=NL, with_logits=True):
    nc = bacc.Bacc(num_devices=NCORES)

    x0_e = nc.declare_dram_parameter("x0", [TL, H], F32, isOutput=False)
    wq_e = nc.declare_dram_parameter("wq", [n_layers, H, H], F16, isOutput=False)
    wk_e = nc.declare_dram_parameter("wk", [n_layers, H, H], F16, isOutput=False)
    wv_e = nc.declare_dram_parameter("wv", [n_layers, H, H], F16, isOutput=False)
    wo_e = nc.declare_dram_parameter("wo", [n_layers, H, H], F16, isOutput=False)
    wf_e = nc.declare_dram_parameter("wf", [n_layers, H, MLP], F16, isOutput=False)
    wp_e = nc.declare_dram_parameter("wp", [n_layers, MLP, H], F16, isOutput=False)
    qb_e = nc.declare_dram_parameter("qb", [n_layers, 128, 8], F32, isOutput=False)
    kb_e = nc.declare_dram_parameter("kb", [n_layers, 128, 8], F32, isOutput=False)
    vb_e = nc.declare_dram_parameter("vb", [n_layers, 1, H], F16, isOutput=False)
    ob_e = nc.declare_dram_parameter("ob", [n_layers, 1, H], F16, isOutput=False)
    fb_e = nc.declare_dram_parameter("fb", [n_layers, 128, 32], F32, isOutput=False)
    pb_e = nc.declare_dram_parameter("pb", [n_layers, 1, H], F16, isOutput=False)
    mg_e = nc.declare_dram_parameter("maskg", [KB, 128, TL], BF16, isOutput=False)
    ml_e = nc.declare_dram_parameter("maskl", [KB, 128, TL], BF16, isOutput=False)
    if with_logits:
        lm_e = nc.declare_dram_parameter("lm", [H, VSH], F16, isOutput=False)
        lbt_e = nc.declare_dram_parameter("lbt", [128, VSH // 128], F32, isOutput=False)
        out_e = nc.declare_dram_parameter("out", [VSH, T], F32, isOutput=True)
    else:
        out_e = nc.declare_dram_parameter("out", [TL, H], F32, isOutput=True)

    from contextlib import ExitStack
    with tile.TileContext(nc) as tc:
        with ExitStack() as _stk:
            _p = lambda *a, **kw: _stk.enter_context(tc.tile_pool(*a, **kw))
            constp = _p(name="const", bufs=1)
            wrowp = _p(name="wrow", bufs=12)    # [128,1024] f16 weight rows
            wsmp = _p(name="wsm", bufs=18)      # [128,128] f16 lhsT blocks (wf, lm)
            wprp = _p(name="wpr", bufs=18)      # [128,512] f16 rhs blocks (wp, xtg)
            ktgp = _p(name="ktg", bufs=6)       # [128,256] f16 gathered k tiles
            vgp = _p(name="vg", bufs=9)         # [128,1024] f32 gathered v tiles
            maskp = _p(name="maskt", bufs=9)    # [128,256] bf16 mask tiles
            xresp = _p(name="xres", bufs=3)     # [128,1024] f32 residual
            hpoolp = _p(name="hpool", bufs=3)   # [128,1024] f16 ln out
            hTp = _p(name="hT", bufs=9)         # [128,256] f16 transposed acts
            qktp = _p(name="qkt", bufs=17)      # [128,256] f16 qT/kT tiles
            vsbp = _p(name="vsb", bufs=4)       # [128,1024]f32 v / [128,512] evicts
            accp = _p(name="acc", bufs=11)      # [128,512] f32 attn accums
            ctxTp = _p(name="ctxT", bufs=9)     # [128,256] f16 ctx
            evp = _p(name="ev", bufs=5)         # [128,256] f32 exp tiles
            gtp = _p(name="gt", bufs=18)        # [128,256] f16 mlp mid
            rbp = _p(name="rb", bufs=3)         # [128,256] f32 recip bcast
            smallp = _p(name="small", bufs=3)
            ps_sc = _p(name="ps_sc", bufs=3, space="PSUM")
            ps_ctx = _p(name="ps_ctx", bufs=2, space="PSUM")
            ps_mm = _p(name="ps_mm", bufs=2, space="PSUM")
            dramp = _p(name="dram", bufs=2, space="DRAM")
            ident = constp.tile([128, 128], F16, name="ident")
            make_identity(nc, ident[:])
            ones_col = constp.tile([128, 32], F32, name="ones_col")
            nc.vector.memset(ones_col[:], 1.0)
            ones_row16 = constp.tile([1, 128], F16, name="ones_row16")
            nc.vector.memset(ones_row16[:], 1.0)
            ones_row32 = constp.tile([1, 128], F32, name="ones_row32")
            nc.vector.memset(ones_row32[:], 1.0)
            eps_t = constp.tile([128, 1], F32, name="eps_t")
            nc.vector.memset(eps_t[:], EPS)

            x_cur = []
            for tt in range(2):
                xt = xresp.tile([128, H], F32, name=f"x_init{tt}", tag="x")
                nc.sync.dma_start(out=xt[:], in_=x0_e[tt * 128:(tt + 1) * 128, :])
                x_cur.append(xt)

            def layernorm_f16(xtiles, nm):
                outs = []
                for tt in range(2):
                    stats = smallp.tile([128, 2, 6], F32, name=f"st{nm}{tt}", tag="st")
                    for s in range(2):
                        nc.vector.bn_stats(out=stats[:, s, :],
                                           in_=xtiles[tt][:, s * 512:(s + 1) * 512])
                    mv = smallp.tile([128, 2], F32, name=f"mv{nm}{tt}", tag="mv")
                    nc.vector.bn_aggr(out=mv[:], in_=stats[:])
                    rstd = smallp.tile([128, 1], F32, name=f"rs{nm}{tt}", tag="rstd")
                    nc.scalar.activation(out=rstd[:], in_=mv[:, 1:2],
                                         func=mybir.ActivationFunctionType.Sqrt,
                                         bias=eps_t[:], scale=1.0)
                    nc.vector.reciprocal(out=rstd[:], in_=rstd[:])
                    h = hpoolp.tile([128, H], F16, name=f"h{nm}{tt}", tag="h")
                    nc.vector.tensor_scalar(out=h[:], in0=xtiles[tt][:],
                                            scalar1=mv[:, 0:1], scalar2=rstd[:],
                                            op0=mybir.AluOpType.subtract,
                                            op1=mybir.AluOpType.mult)
                    outs.append(h)
                return outs

            def transpose_h(htiles, nm):
                hT = []
                for hk in range(8):
                    t = hTp.tile([128, TL], F16, name=f"hT{nm}{hk}", tag="hT")
                    for tt in range(2):
                        pt = ps_sc.tile([128, 128], F16, name=f"ptr{nm}{hk}{tt}", tag="sc")
                        nc.tensor.transpose(pt[:], htiles[tt][:, hk * 128:(hk + 1) * 128],
                                            ident[:])
                        nc.vector.tensor_copy(out=t[:, tt * 128:(tt + 1) * 128], in_=pt[:])
                    hT.append(t)
                return hT

            def load_wrows(we, l, nm):
                rows = []
                for k in range(8):
                    w = wrowp.tile([128, H], F16, name=f"{nm}{l}_{k}", tag="wrow")
                    nc.sync.dma_start(out=w[:], in_=we[l, k * 128:(k + 1) * 128, :])
                    rows.append(w)
                return rows

            for l in range(n_layers):
                mask_e = ml_e if ATTN_LOCAL[l] else mg_e

                h1 = layernorm_f16(x_cur, f"l{l}a")
                hT = transpose_h(h1, f"l{l}a")

                qb_sb = smallp.tile([128, 8], F32, name=f"qb{l}", tag="qb")
                nc.sync.dma_start(out=qb_sb[:], in_=qb_e[l])
                kb_sb = smallp.tile([128, 8], F32, name=f"kb{l}", tag="kb")
                nc.sync.dma_start(out=kb_sb[:], in_=kb_e[l])
                vb_sb = smallp.tile([1, H], F16, name=f"vb{l}", tag="vb")
                nc.sync.dma_start(out=vb_sb[:], in_=vb_e[l])
                ob_sb = smallp.tile([1, H], F16, name=f"ob{l}", tag="ob")
                nc.sync.dma_start(out=ob_sb[:], in_=ob_e[l])
                fb_sb = smallp.tile([128, 32], F32, name=f"fb{l}", tag="fb")
                nc.sync.dma_start(out=fb_sb[:], in_=fb_e[l])
                pb_sb = smallp.tile([1, H], F16, name=f"pb{l}", tag="pb")
                nc.sync.dma_start(out=pb_sb[:], in_=pb_e[l])

                # ---- kT first so AllGather(k) overlaps v/q compute ----
                wkr = load_wrows(wk_e, l, "wk")
                bounce_k = dramp.tile([H, TL], F16, name=f"bk{l}", tag="bk")
                for of in range(8):
                    pq = ps_sc.tile([128, TL], F32, name=f"pk{l}{of}", tag="sc")
                    for k in range(8):
                        nc.tensor.matmul(pq[:], wkr[k][:, of * 128:(of + 1) * 128], hT[k][:],
                                         start=(k == 0), stop=(k == 7))
                    t = qktp.tile([128, TL], F16, name=f"kt{l}{of}", tag="qkt")
                    nc.vector.tensor_scalar_add(out=t[:], in0=pq[:],
                                                scalar1=kb_sb[:, of:of + 1])
                    nc.sync.dma_start(out=bounce_k[of * 128:(of + 1) * 128, :], in_=t[:])
                gath_k = dramp.tile([NCORES * H, TL], F16, name=f"gk{l}", tag="gk",
                                    addr_space="Shared")
                nc.gpsimd.collective_compute("AllGather", mybir.AluOpType.bypass,
                                             replica_groups=RG,
                                             ins=[bounce_k[:]], outs=[gath_k[:]])

                # ---- v (f32 out, fp32 ctx matmul needs f32 operands) ----
                wvr = load_wrows(wv_e, l, "wv")
                bounce_v = dramp.tile([TL, H], F32, name=f"bv{l}", tag="bv")
                for tt in range(2):
                    vt = vsbp.tile([128, H], F32, name=f"v{l}{tt}", tag="vsb")
                    for nn in range(2):
                        pv = ps_mm.tile([128, 512], F32, name=f"pv{l}{tt}{nn}", tag="mm")
                        for k in range(8):
                            nc.tensor.matmul(pv[:], hT[k][:, tt * 128:(tt + 1) * 128],
                                             wvr[k][:, nn * 512:(nn + 1) * 512],
                                             start=(k == 0), stop=False)
                        nc.tensor.matmul(pv[:], ones_row16[:, 0:128],
                                         vb_sb[:, nn * 512:(nn + 1) * 512],
                                         start=False, stop=True)
                        nc.vector.tensor_copy(out=vt[:, nn * 512:(nn + 1) * 512], in_=pv[:])
                    nc.sync.dma_start(out=bounce_v[tt * 128:(tt + 1) * 128, :], in_=vt[:])
                gath_v = dramp.tile([T, H], F32, name=f"gv{l}", tag="gv", addr_space="Shared")
                nc.gpsimd.collective_compute("AllGather", mybir.AluOpType.bypass,
                                             replica_groups=RG,
                                             ins=[bounce_v[:]], outs=[gath_v[:]])

                # ---- qT (stays local) ----
                wqr = load_wrows(wq_e, l, "wq")
                qt = []
                for of in range(8):
                    pq = ps_sc.tile([128, TL], F32, name=f"pq{l}{of}", tag="sc")
                    for k in range(8):
                        nc.tensor.matmul(pq[:], wqr[k][:, of * 128:(of + 1) * 128], hT[k][:],
                                         start=(k == 0), stop=(k == 7))
                    t = qktp.tile([128, TL], F16, name=f"qt{l}{of}", tag="qkt")
                    nc.vector.tensor_scalar_add(out=t[:], in0=pq[:],
                                                scalar1=qb_sb[:, of:of + 1])
                    qt.append(t)

                # ---- attention: kb-half outer, head-pair inner ----
                acc_t = [None] * HP
                ctxT = [None] * HP
                for half in range(2):
                    kbs = range(half * 8, half * 8 + 8)
                    vg = {}
                    for kb in kbs:
                        vt = vgp.tile([128, H], F32, name=f"vg{l}{kb}", tag="vg")
                        nc.sync.dma_start(out=vt[:], in_=gath_v[kb * 128:(kb + 1) * 128, :])
                        vg[kb] = vt
                    mt = {}
                    for kb in kbs:
                        m = maskp.tile([128, TL], BF16, name=f"m{l}{kb}", tag="mask")
                        nc.sync.dma_start(out=m[:], in_=mask_e[kb])
                        mt[kb] = m
                    for hp in range(HP):
                        ktg = {}
                        for cc in range(half * 4, half * 4 + 4):
                            t = ktgp.tile([128, TL], F16, name=f"ktg{l}{hp}{cc}", tag="ktg")
                            nc.sync.dma_start(
                                out=t[:],
                                in_=gath_k[cc * H + hp * 128: cc * H + (hp + 1) * 128, :])
                            ktg[cc] = t
                        pcs = ps_ctx.tile([128, 512], F32, name=f"pcs{l}{half}{hp}", tag="ctx")
                        # interleaved accumulation groups share this bank; a
                        # start=True would mark the whole 2KB bank row pending-
                        # zero and wipe sibling groups, so init via memset and
                        # accumulate with start=False throughout.
                        nc.vector.memset(pcs[:], 0.0)
                        for kb in kbs:
                            cc, hf = kb // 2, kb % 2
                            colsl = slice(hf * 128, (hf + 1) * 128)
                            s0 = ps_sc.tile([128, TL], F32, name=f"s0_{l}{hp}{kb}", tag="sc")
                            s1 = ps_sc.tile([128, TL], F32, name=f"s1_{l}{hp}{kb}", tag="sc")
                            nc.tensor.matmul(s0[:], ktg[cc][0:64, colsl], qt[hp][0:64, :],
                                             start=True, stop=True)
                            nc.tensor.matmul(s1[:], ktg[cc][64:128, colsl], qt[hp][64:128, :],
                                             start=True, stop=True)
                            e0 = evp.tile([128, TL], F32, name=f"e0_{l}{hp}{kb}", tag="ev")
                            e1 = evp.tile([128, TL], F32, name=f"e1_{l}{hp}{kb}", tag="ev")
                            nc.vector.tensor_tensor(out=e0[:], in0=s0[:], in1=mt[kb][:],
                                                    op=mybir.AluOpType.add)
                            nc.vector.tensor_tensor(out=e1[:], in0=s1[:], in1=mt[kb][:],
                                                    op=mybir.AluOpType.add)
                            nc.scalar.activation(out=e0[:], in_=e0[:],
                                                 func=mybir.ActivationFunctionType.Exp)
                            nc.scalar.activation(out=e1[:], in_=e1[:],
                                                 func=mybir.ActivationFunctionType.Exp)
                            sp = (kb == half * 8 + 7)
                            nc.tensor.matmul(pcs[0:64, 0:TL],
                                             vg[kb][:, hp * 128:hp * 128 + 64], e0[:],
                                             start=False, stop=sp, tile_position=(0, 0),
                                             skip_group_check=True)
                            nc.tensor.matmul(pcs[64:128, 0:TL],
                                             vg[kb][:, hp * 128 + 64:(hp + 1) * 128], e1[:],
                                             start=False, stop=sp, tile_position=(0, 64),
                                             skip_group_check=True)
                            nc.tensor.matmul(pcs[0:32, TL:2 * TL], ones_col[:], e0[:],
                                             start=False, stop=sp, tile_position=(0, 0),
                                             skip_group_check=True)
                            nc.tensor.matmul(pcs[32:64, TL:2 * TL], ones_col[:], e1[:],
                                             start=False, stop=sp, tile_position=(0, 32),
                                             skip_group_check=True)
                        if half == 0:
                            a = accp.tile([128, 512], F32, name=f"ac{l}{hp}", tag="acc")
                            nc.vector.tensor_copy(out=a[:], in_=pcs[:])
                            acc_t[hp] = a
                        else:
                            comb = accp.tile([128, 512], F32, name=f"cb{l}{hp}", tag="acc")
                            nc.vector.tensor_tensor(out=comb[:], in0=pcs[:],
                                                    in1=acc_t[hp][:],
                                                    op=mybir.AluOpType.add)
                            rsA = smallp.tile([1, TL], F32, name=f"rsA{l}{hp}", tag="rsA")
                            rsB = smallp.tile([1, TL], F32, name=f"rsB{l}{hp}", tag="rsB")
                            nc.vector.reciprocal(out=rsA[:], in_=comb[0:1, TL:2 * TL])
                            nc.vector.reciprocal(out=rsB[:], in_=comb[32:33, TL:2 * TL])
                            pbc = ps_sc.tile([128, TL], F32, name=f"pbc{l}{hp}", tag="sc")
                            nc.tensor.matmul(pbc[0:64, :], ones_row32[:, 0:64], rsA[:],
                                             start=True, stop=True, tile_position=(0, 0))
                            nc.tensor.matmul(pbc[64:128, :], ones_row32[:, 0:64], rsB[:],
                                             start=True, stop=True, tile_position=(0, 64))
                            rb = rbp.tile([128, TL], F32, name=f"rb{l}{hp}", tag="rb")
                            nc.vector.tensor_copy(out=rb[:], in_=pbc[:])
                            ct = ctxTp.tile([128, TL], F16, name=f"ct{l}{hp}", tag="ctxT")
                            nc.vector.tensor_tensor(out=ct[:], in0=comb[:, 0:TL], in1=rb[:],
                                                    op=mybir.AluOpType.mult)
                            ctxT[hp] = ct

                # ---- attention out projection + residual ----
                wor = load_wrows(wo_e, l, "wo")
                x_new = []
                for tt in range(2):
                    xt = xresp.tile([128, H], F32, name=f"xa{l}{tt}", tag="x")
                    for nn in range(2):
                        pa = ps_mm.tile([128, 512], F32, name=f"pa{l}{tt}{nn}", tag="mm")
                        for k in range(8):
                            nc.tensor.matmul(pa[:], ctxT[k][:, tt * 128:(tt + 1) * 128],
                                             wor[k][:, nn * 512:(nn + 1) * 512],
                                             start=(k == 0), stop=False)
                        nc.tensor.matmul(pa[:], ones_row16[:, 0:128],
                                         ob_sb[:, nn * 512:(nn + 1) * 512],
                                         start=False, stop=True)
                        nc.vector.tensor_tensor(out=xt[:, nn * 512:(nn + 1) * 512],
                                                in0=pa[:],
                                                in1=x_cur[tt][:, nn * 512:(nn + 1) * 512],
                                                op=mybir.AluOpType.add)
                    x_new.append(xt)
                x_cur = x_new

                # ---- MLP (two halves of the 4096 dim) ----
                h2 = layernorm_f16(x_cur, f"l{l}b")
                h2T = transpose_h(h2, f"l{l}b")
                x_new = [xresp.tile([128, H], F32, name=f"xm{l}{tt}", tag="x")
                         for tt in range(2)]
                part_sb = [[None, None], [None, None]]
                for halfk in range(2):
                    gts = []
                    for ofh in range(16):
                        of = halfk * 16 + ofh
                        # fc weight lhsT blocks [128, 128] for this of
                        wfb = []
                        for k in range(8):
                            w = wsmp.tile([128, 128], F16, name=f"wf{l}{of}{k}", tag="wsm")
                            nc.sync.dma_start(out=w[:], in_=wf_e[l, k * 128:(k + 1) * 128,
                                                              of * 128:(of + 1) * 128])
                            wfb.append(w)
                        pf = ps_sc.tile([128, TL], F32, name=f"pf{l}{of}", tag="sc")
                        for k in range(8):
                            nc.tensor.matmul(pf[:], wfb[k][:], h2T[k][:],
                                             start=(k == 0), stop=(k == 7))
                        g = gtp.tile([128, TL], F16, name=f"g{l}{of}", tag="g")
                        nc.scalar.activation(out=g[:], in_=pf[:],
                                             func=mybir.ActivationFunctionType.Gelu,
                                             bias=fb_sb[:, of:of + 1], scale=1.0)
                        gts.append(g)
                    for nn in range(2):
                        wpr = []
                        for kk in range(16):
                            k_of = halfk * 16 + kk
                            w = wprp.tile([128, 512], F16, name=f"wp{l}{k_of}{nn}", tag="wpr")
                            nc.sync.dma_start(out=w[:], in_=wp_e[l, k_of * 128:(k_of + 1) * 128,
                                                              nn * 512:(nn + 1) * 512])
                            wpr.append(w)
                        for tt in range(2):
                            pp = ps_mm.tile([128, 512], F32, name=f"pp{l}{halfk}{tt}{nn}",
                                            tag="mm")
                            for kk in range(16):
                                nc.tensor.matmul(pp[:], gts[kk][:, tt * 128:(tt + 1) * 128],
                                                 wpr[kk][:],
                                                 start=(kk == 0),
                                                 stop=(halfk == 0 and kk == 15))
                            if halfk == 0:
                                s = accp.tile([128, 512], F32, name=f"ph{l}{tt}{nn}",
                                              tag="acc")
                                nc.vector.tensor_copy(out=s[:], in_=pp[:])
                                part_sb[tt][nn] = s
                            else:
                                nc.tensor.matmul(pp[:], ones_row16[:, 0:128],
                                                 pb_sb[:, nn * 512:(nn + 1) * 512],
                                                 start=False, stop=True)
                                t2 = vsbp.tile([128, 512], F32, name=f"pj{l}{tt}{nn}",
                                               tag="vsb")
                                nc.vector.tensor_tensor(out=t2[:], in0=pp[:],
                                                        in1=part_sb[tt][nn][:],
                                                        op=mybir.AluOpType.add)
                                nc.vector.tensor_tensor(
                                    out=x_new[tt][:, nn * 512:(nn + 1) * 512],
                                    in0=t2[:],
                                    in1=x_cur[tt][:, nn * 512:(nn + 1) * 512],
                                    op=mybir.AluOpType.add)
                x_cur = x_new

            if not with_logits:
                for tt in range(2):
                    nc.sync.dma_start(out=out_e[tt * 128:(tt + 1) * 128, :], in_=x_cur[tt][:])
            else:
                xh = layernorm_f16(x_cur, "f")
                xhT = transpose_h(xh, "f")
                bounce_x = dramp.tile([H, TL], F16, name="bx", tag="bx")
                for hk in range(8):
                    nc.sync.dma_start(out=bounce_x[hk * 128:(hk + 1) * 128, :], in_=xhT[hk][:])
                gath_x = dramp.tile([NCORES * H, TL], F16, name="gx", tag="gx",
                                    addr_space="Shared")
                nc.gpsimd.collective_compute("AllGather", mybir.AluOpType.bypass,
                                             replica_groups=RG,
                                             ins=[bounce_x[:]], outs=[gath_x[:]])
                lbt_sb = smallp.tile([128, VSH // 128], F32, name="lbt", tag="lbt")
                nc.sync.dma_start(out=lbt_sb[:], in_=lbt_e[:])
                # logitsT[vv*128:(vv+1)*128, :] = lm_tile.T @ xT
                for tc4 in range(4):
                    xtgc = []
                    for k in range(8):
                        t = wprp.tile([128, 512], F16, name=f"xtg{tc4}{k}", tag="wpr")
                        for j in range(2):
                            cc = tc4 * 2 + j
                            nc.sync.dma_start(
                                out=t[:, j * TL:(j + 1) * TL],
                                in_=gath_x[cc * H + k * 128: cc * H + (k + 1) * 128, :])
                        xtgc.append(t)
                    for vv in range(VSH // 128):
                        lmt = []
                        for k in range(8):
                            t = wsmp.tile([128, 128], F16, name=f"lm{tc4}{vv}{k}", tag="wsm")
                            nc.sync.dma_start(out=t[:], in_=lm_e[k * 128:(k + 1) * 128,
                                                              vv * 128:(vv + 1) * 128])
                            lmt.append(t)
                        pl = ps_mm.tile([128, 512], F32, name=f"pl{tc4}{vv}", tag="mm")
                        for k in range(8):
                            nc.tensor.matmul(pl[:], lmt[k][:], xtgc[k][:],
                                             start=(k == 0), stop=(k == 7))
                        o = vsbp.tile([128, 512], F32, name=f"o{tc4}{vv}", tag="vsb")
                        nc.vector.tensor_scalar_add(out=o[:], in0=pl[:],
                                                    scalar1=lbt_sb[:, vv:vv + 1])
                        nc.sync.dma_start(out=out_e[vv * 128:(vv + 1) * 128,
                                                    tc4 * 512:(tc4 + 1) * 512], in_=o[:])

    nc.finalize()
    return nc


# ------------------- host-side prep & entry -------------------

def _prep_inputs(inputs, n_layers=NL, with_logits=True):
    f32 = np.float32
    f16 = np.float16
    import ml_dtypes
    bf16 = ml_dtypes.bfloat16

    ids = np.asarray(inputs["input_ids"]).reshape(-1).astype(np.int64)
    wte = np.asarray(inputs["wte"], f32)
    wpe = np.asarray(inputs["wpe"], f32)
    x0 = wte[ids] + wpe[:T]

    wq = np.empty((n_layers, H, H), f16); wk = np.empty((n_layers, H, H), f16)
    wv = np.empty((n_layers, H, H), f16); wo = np.empty((n_layers, H, H), f16)
    wf = np.empty((n_layers, H, MLP), f16); wp = np.empty((n_layers, MLP, H), f16)
    qb = np.empty((n_layers, 128, 8), f32); kbb = np.empty((n_layers, 128, 8), f32)
    vb = np.empty((n_layers, 1, H), f16); ob = np.empty((n_layers, 1, H), f16)
    fb = np.empty((n_layers, 128, 32), f32); pb = np.empty((n_layers, 1, H), f16)
    for l in range(n_layers):
        ln1w = np.asarray(inputs["ln1_w"][l], f32); ln1b = np.asarray(inputs["ln1_b"][l], f32)
        ln2w = np.asarray(inputs["ln2_w"][l], f32); ln2b = np.asarray(inputs["ln2_b"][l], f32)
        for (wdst, bdst, wname) in ((wq, qb, "q_w"), (wk, kbb, "k_w")):
            w = np.asarray(inputs[wname][l], f32)
            wdst[l] = (ln1w[:, None] * w).astype(f16)
            bdst[l] = (ln1b @ w).reshape(8, 128).T
        w = np.asarray(inputs["v_w"][l], f32)
        wv[l] = (ln1w[:, None] * w).astype(f16)
        vb[l] = (ln1b @ w)[None, :].astype(f16)
        wo[l] = np.asarray(inputs["o_w"][l], f32).astype(f16)
        ob[l] = np.asarray(inputs["o_b"][l], f32)[None, :].astype(f16)
        w = np.asarray(inputs["fc_w"][l], f32)
        wf[l] = (ln2w[:, None] * w).astype(f16)
        fbv = np.asarray(inputs["fc_b"][l], f32) + ln2b @ w
        fb[l] = fbv.reshape(32, 128).T
        wp[l] = np.asarray(inputs["proj_w"][l], f32).astype(f16)
        pb[l] = np.asarray(inputs["proj_b"][l], f32)[None, :].astype(f16)

    lnfw = np.asarray(inputs["lnf_w"], f32); lnfb = np.asarray(inputs["lnf_b"], f32)
    VP = NCORES * VSH
    lm_pad = np.zeros((VP, H), f16)
    lm_pad[:VOCAB] = (wte * lnfw[None, :]).astype(f16)
    lb_pad = np.zeros((VP,), f32)
    lb_pad[:VOCAB] = wte @ lnfb

    in_maps = []
    for c in range(NCORES):
        ts = c * TL
        qi = ts + np.arange(TL)[None, :]
        kj = np.arange(128)[:, None]
        mg = np.empty((KB, 128, TL), bf16)
        mlm = np.empty((KB, 128, TL), bf16)
        for kb in range(KB):
            ka = kb * 128 + kj
            causal = (ka <= qi)
            mg[kb] = np.where(causal, 0.0, -30000.0).astype(bf16)
            mlm[kb] = np.where(causal & (qi - ka < WINDOW), 0.0, -30000.0).astype(bf16)
        m = {
            "x0": np.ascontiguousarray(x0[ts:ts + TL]).astype(f32),
            "wq": wq, "wk": wk, "wv": wv, "wo": wo, "wf": wf, "wp": wp,
            "qb": qb, "kb": kbb, "vb": vb, "ob": ob, "fb": fb, "pb": pb,
            "maskg": mg, "maskl": mlm,
        }
        if with_logits:
            m["lm"] = np.ascontiguousarray(lm_pad[c * VSH:(c + 1) * VSH].T)
            m["lbt"] = np.ascontiguousarray(
                lb_pad[c * VSH:(c + 1) * VSH].reshape(VSH // 128, 128).T)
        in_maps.append(m)
    return in_maps


_NC_CACHE = {}


def _get_nc(n_layers=NL, with_logits=True):
    key = (n_layers, with_logits)
    if key not in _NC_CACHE:
        _NC_CACHE[key] = build(n_layers, with_logits)
    return _NC_CACHE[key]


def run(inputs, n_layers=NL, with_logits=True, trace=False):
    nc = _get_nc(n_layers, with_logits)
    in_maps = _prep_inputs(inputs, n_layers, with_logits)
    res = run_bass_kernel_spmd(nc, in_maps, list(range(NCORES)), trace=trace)
    if with_logits:
        parts = [res.results[c]["out"] for c in range(NCORES)]   # each [VSH, T]
        full = np.concatenate(parts, axis=0)[:VOCAB]             # [VOCAB, T]
        out = np.ascontiguousarray(full.T)[None]                 # [1, T, VOCAB]
    else:
        out = np.concatenate([res.results[c]["out"] for c in range(NCORES)], axis=0)[None]
    return out, res


def kernel(**inputs) -> np.ndarray:
    out, _ = run(inputs, NL, True, trace=False)
    return out

